# revision 11
# baseline (speedup 1.0000x reference)
"""Trainium2 Bass kernel: AdjacencyLearn GNN (encoder + gumbel softmax + GRU decoder).

Sharding: data-parallel over batch B=16 across 8 NeuronCores (2 batch/core).
Activations feature-major [feature on partitions, rows on free]. node->edge
gather and edge->node scatter-add run as PE matmuls against host-built one-hot
matrices. BatchNorm training statistics stay exact via [128,2] AllReduces.
"""

import numpy as np

B, N, T, D = 16, 40, 20, 4
H, K = 128, 2
E = N * (N - 1)
TAU, GEPS, BN_EPS = 0.5, 1e-10, 1e-5
NC = 8
BL = B // NC            # 2
R = BL * N              # 80
EL = BL * E             # 3120
NCH, CH = 8, EL // 8    # 8 chunks x 390
NT = (EL + 127) // 128  # 25 edge tiles
TD = T * D              # 80
TS = T - 1              # 19

CV = dict(m1b1=0, m1b2=1, m2b1=2, m2b2=3, m3b1=4, m3b2=5, m4b1=6, m4b2=7,
          g1=8, bb1=9, g2=10, bb2=11, g3=12, bb3=13, g4=14, bb4=15,
          msgb1=16, bir=17, bii=18, bin=19, bof1=20, bof2=21, dwt=22,
          neg1=23, bneps=24, geps=25)
WM = dict(m1w2=0, m2a=1, m2b=2, m2w2=3, m3w1=4, m3w2=5, m4a=6, m4b=7, m4c=8,
          m4w2=9, msga=10, msgb=11, msgw2=12, hr=13, hi=14, hn=15, of1=16, of2=17)
NWM = 18

_CACHE = {}


def _f32(x):
    return np.ascontiguousarray(np.asarray(x), dtype=np.float32)


def _build():
    import contextlib
    import concourse.mybir as mybir
    import concourse.tile as tile
    from concourse import bacc
    from concourse.masks import make_identity

    dt = mybir.dt
    AF = mybir.ActivationFunctionType
    AL = mybir.AluOpType
    AX = mybir.AxisListType

    nc = bacc.Bacc("TRN2", target_bir_lowering=False, debug=False, num_devices=NC)

    def din(name, shape):
        return nc.dram_tensor(name, list(shape), dt.float32, kind="ExternalInput")

    x1_in = din("x1", [TD, R])
    ins_in = din("insall", [D, TS * R])
    gum_in = din("gum", [128, NT, 2])
    grec_in = din("grec", [R, EL])
    gsend_in = din("gsend", [R, EL])
    s01n_in = din("s01n", [128, NT, R])
    s01d_in = din("s01d", [128, NT, R])
    cvec_in = din("cvec", [128, 27])
    wmat_in = din("wmat", [128, NWM * 128])
    wm1_in = din("wm1", [TD, 128])
    wir_in = din("wir", [D, 3 * 128])
    wof3_in = din("wof3", [128, D])
    bof3_in = din("bof3", [D, 1])
    b2row_in = din("b2row", [1, 128])
    db_in = 0.0  # delta fc bias folded on host into sigmoid bias (see below)
    out_t = nc.dram_tensor("preds", [D, TS * R], dt.float32, kind="ExternalOutput")

    uid = [0]

    def nm(p):
        uid[0] += 1
        return f"{p}{uid[0]}"

    with tile.TileContext(nc) as tc:
        ctx = contextlib.ExitStack()
        const = ctx.enter_context(tc.tile_pool(name="const", bufs=1))
        big = ctx.enter_context(tc.tile_pool(name="big", bufs=1))
        bige = ctx.enter_context(tc.tile_pool(name="bige", bufs=1))
        work = ctx.enter_context(tc.tile_pool(name="work", bufs=3))
        small = ctx.enter_context(tc.tile_pool(name="small", bufs=4))
        ps = ctx.enter_context(tc.tile_pool(name="ps", bufs=3, space="PSUM"))
        pse = ctx.enter_context(tc.tile_pool(name="pse", bufs=2, space="PSUM"))
        dram = ctx.enter_context(tc.tile_pool(name="dram", bufs=2, space="DRAM"))

        cvec = const.tile_from(cvec_in[:])
        wmat = const.tile_from(wmat_in[:])
        wm1 = const.tile_from(wm1_in[:])
        wir = const.tile_from(wir_in[:])
        wof3 = const.tile_from(wof3_in[:])
        bof3 = const.tile_from(bof3_in[:])
        b2row = const.tile_from(b2row_in[:])
        grec = const.tile_from(grec_in[:])
        gsend = const.tile_from(gsend_in[:])
        s01n = const.tile_from(s01n_in[:])
        srel = const.tile_from(s01d_in[:])
        gum = const.tile_from(gum_in[:])
        x1 = const.tile_from(x1_in[:])
        insall = const.tile_from(ins_in[:])
        ident = const.tile([128, 128], dt.float32, name="ident")
        make_identity(nc, ident)
        ones1 = const.tile([1, 128], dt.float32, name="ones1")
        nc.vector.memset(ones1[:], 1.0)

        def cv(n):
            return cvec[:, CV[n]:CV[n] + 1]

        def wm(n):
            o = WM[n] * 128
            return wmat[:, o:o + 128]

        def tsz(t):
            return min(128, EL - t * 128)

        def allreduce_bn(x_sb, nrows, gname, bname, tag):
            stats = small.tile([128, 2], dt.float32, name=nm("st"), tag="st")
            sq = work.tile([128, nrows], dt.float32, name=nm("sq"), tag="sq", bufs=1)
            nc.scalar.activation(sq[:], x_sb, AF.Square)
            nc.vector.tensor_reduce(stats[:, 0:1], x_sb, AX.X, AL.add)
            nc.vector.tensor_reduce(stats[:, 1:2], sq[:], AX.X, AL.add)
            dri = dram.tile([128, 2], dt.float32, name=nm("dri"))
            dro = dram.tile([128, 2], dt.float32, name=nm("dro"))
            nc.gpsimd.dma_start(dri[:], stats[:])
            nc.gpsimd.collective_compute(
                "AllReduce", AL.add, replica_groups=[list(range(NC))],
                ins=[dri.opt()], outs=[dro.opt()])
            tot = small.tile([128, 2], dt.float32, name=nm("tot"), tag="tot")
            nc.sync.dma_start(tot[:], dro[:])
            m = small.tile([128, 1], dt.float32, name=nm("m"), tag="m")
            v = small.tile([128, 1], dt.float32, name=nm("v"), tag="v")
            s = small.tile([128, 1], dt.float32, name=nm("s"), tag="s")
            c = small.tile([128, 1], dt.float32, name=nm("c"), tag="c")
            inv = 1.0 / (nrows * NC)
            nc.vector.tensor_scalar_mul(m[:], tot[:, 0:1], inv)
            nc.vector.tensor_scalar_mul(v[:], tot[:, 1:2], inv)
            msq = small.tile([128, 1], dt.float32, name=nm("msq"), tag="msq")
            nc.scalar.activation(msq[:], m[:], AF.Square)
            nc.vector.tensor_tensor(v[:], v[:], msq[:], AL.subtract)
            nc.scalar.activation(v[:], v[:], AF.Sqrt, bias=cv("bneps"))
            nc.vector.reciprocal(v[:], v[:])
            nc.vector.tensor_tensor(s[:], v[:], cv(gname), AL.mult)
            nc.vector.tensor_tensor(c[:], m[:], s[:], AL.mult)
            nc.vector.tensor_tensor(c[:], cv(bname), c[:], AL.subtract)
            return s, c

        def elu_p1(dst_ap, psum_ap, bcol, nrows):
            # dst = elu(psum + bcol - 1) + 1   (bcol = true_bias + 1)
            hp = work.tile([128, nrows], dt.float32, name=nm("hp"), tag="hp")
            nc.scalar.activation(hp[:], psum_ap, AF.Identity, bias=bcol)
            xm = work.tile([128, nrows], dt.float32, name=nm("xm"), tag="xm")
            nc.vector.tensor_scalar_min(xm[:], hp[:], 1.0)
            nc.scalar.activation(xm[:], xm[:], AF.Exp, bias=cv("neg1"))
            nc.vector.tensor_tensor(dst_ap, xm[:], hp[:], AL.max)

        # ================= encoder =================
        p = ps.tile([128, R], dt.float32, name=nm("p"), tag="pgen")
        nc.tensor.matmul(p[:], wm1[:], x1[:], start=True, stop=True)
        e1 = work.tile([128, R], dt.float32, name=nm("e1"), tag="en")
        elu_p1(e1[:], p[:], cv("m1b1"), R)
        p2 = ps.tile([128, R], dt.float32, name=nm("p2"), tag="pgen")
        nc.tensor.matmul(p2[:], wm("m1w2"), e1[:], start=True, stop=True)
        e2 = work.tile([128, R], dt.float32, name=nm("e2"), tag="en2")
        elu_p1(e2[:], p2[:], cv("m1b2"), R)
        s1, c1 = allreduce_bn(e2[:], R, "g1", "bb1", "bn1")
        xnode = big.tile([128, R], dt.float32, name="xnode")
        nc.vector.tensor_scalar(xnode[:], e2[:], s1[:], c1[:], AL.mult, AL.add)

        puv = ps.tile([R, 256], dt.float32, name=nm("puv"), tag="pgen")
        nc.tensor.matmul(puv[:], xnode[:], wmat[:, WM["m2a"] * 128:(WM["m2b"] + 1) * 128],
                         start=True, stop=True)
        uv2 = big.tile([R, 256], dt.float32, name="uv2")
        nc.vector.tensor_copy(uv2[:], puv[:])

        xedge = big.tile([128, EL], dt.float32, name="xedge")
        for ci in range(NCH):
            c0 = ci * CH
            pe = pse.tile([128, CH], dt.float32, name=nm("pe"), tag="pe")
            nc.tensor.matmul(pe[:], uv2[:, 0:128], grec[:, c0:c0 + CH], start=True, stop=False)
            nc.tensor.matmul(pe[:], uv2[:, 128:256], gsend[:, c0:c0 + CH], start=False, stop=True)
            ec = work.tile([128, CH], dt.float32, name=nm("ec"), tag="ec")
            elu_p1(ec[:], pe[:], cv("m2b1"), CH)
            pe2 = pse.tile([128, CH], dt.float32, name=nm("pe2"), tag="pe2")
            nc.tensor.matmul(pe2[:], wm("m2w2"), ec[:], start=True, stop=True)
            elu_p1(xedge[:, c0:c0 + CH], pe2[:], cv("m2b2"), CH)
        s2, c2 = allreduce_bn(xedge[:], EL, "g2", "bb2", "bn2")
        nc.vector.tensor_scalar(xedge[:], xedge[:], s2[:], c2[:], AL.mult, AL.add)  # xs in place

        xsT = big.tile([128, NT, 128], dt.float32, name="xsT")
        for t in range(NT):
            z = tsz(t)
            pt = ps.tile([128, 128], dt.float32, name=nm("pt"), tag="pgen")
            nc.tensor.transpose(pt[:z, :], xedge[:, t * 128:t * 128 + z], ident[:])
            nc.vector.tensor_copy(xsT[:z, t, :], pt[:z, :])
        pn = ps.tile([128, R], dt.float32, name=nm("pn"), tag="pgen")
        for t in range(NT):
            z = tsz(t)
            nc.tensor.matmul(pn[:], xsT[:z, t, :], s01n[:z, t, :],
                             start=(t == 0), stop=(t == NT - 1))
        nodes = work.tile([128, R], dt.float32, name=nm("nodes"), tag="en")
        nc.vector.tensor_copy(nodes[:], pn[:])

        p3 = ps.tile([128, R], dt.float32, name=nm("p3"), tag="pgen")
        nc.tensor.matmul(p3[:], wm("m3w1"), nodes[:], start=True, stop=True)
        e31 = work.tile([128, R], dt.float32, name=nm("e31"), tag="en2")
        elu_p1(e31[:], p3[:], cv("m3b1"), R)
        p3b = ps.tile([128, R], dt.float32, name=nm("p3b"), tag="pgen")
        nc.tensor.matmul(p3b[:], wm("m3w2"), e31[:], start=True, stop=True)
        e32 = work.tile([128, R], dt.float32, name=nm("e32"), tag="en")
        elu_p1(e32[:], p3b[:], cv("m3b2"), R)
        s3, c3 = allreduce_bn(e32[:], R, "g3", "bb3", "bn3")
        x3 = big.tile([128, R], dt.float32, name="x3")
        nc.vector.tensor_scalar(x3[:], e32[:], s3[:], c3[:], AL.mult, AL.add)

        puv4 = ps.tile([R, 256], dt.float32, name=nm("puv4"), tag="pgen")
        nc.tensor.matmul(puv4[:], x3[:], wmat[:, WM["m4a"] * 128:(WM["m4b"] + 1) * 128],
                         start=True, stop=True)
        uv4 = big.tile([R, 256], dt.float32, name="uv4")
        nc.vector.tensor_copy(uv4[:], puv4[:])

        x4 = big.tile([128, EL], dt.float32, name="x4")
        for ci in range(NCH):
            c0 = ci * CH
            pe = pse.tile([128, CH], dt.float32, name=nm("pe4"), tag="pe")
            nc.tensor.matmul(pe[:], uv4[:, 0:128], grec[:, c0:c0 + CH], start=True, stop=False)
            nc.tensor.matmul(pe[:], uv4[:, 128:256], gsend[:, c0:c0 + CH], start=False, stop=False)
            nc.tensor.matmul(pe[:], wm("m4c"), xedge[:, c0:c0 + CH], start=False, stop=True)
            ec = work.tile([128, CH], dt.float32, name=nm("ec4"), tag="ec")
            elu_p1(ec[:], pe[:], cv("m4b1"), CH)
            pe2 = pse.tile([128, CH], dt.float32, name=nm("pe42"), tag="pe2")
            nc.tensor.matmul(pe2[:], wm("m4w2"), ec[:], start=True, stop=True)
            elu_p1(x4[:, c0:c0 + CH], pe2[:], cv("m4b2"), CH)
        s4, c4 = allreduce_bn(x4[:], EL, "g4", "bb4", "bn4")
        nc.vector.tensor_scalar(x4[:], x4[:], s4[:], c4[:], AL.mult, AL.add)

        # ---- rel_type (edge-major wrapped [128, NT]) ----
        pz = ps.tile([128, NT], dt.float32, name="pz", tag="pgen")
        for t in range(NT):
            z = tsz(t)
            nc.tensor.matmul(pz[:z, t:t + 1], x4[:, t * 128:t * 128 + z], cv("dwt"),
                             start=True, stop=True, skip_group_check=True)
        lg = work.tile([128, NT * 2], dt.float32, name="lg", tag="lg")
        gumf = gum[:].rearrange("p a b -> p (a b)")
        nc.scalar.activation(lg[:], gumf, AF.Ln, bias=cv("geps"))
        nc.scalar.activation(lg[:], lg[:], AF.Ln, bias=cv("geps"), scale=-1.0)
        lg3 = lg[:].rearrange("p (a b) -> p a b", b=2)
        dg = work.tile([128, NT], dt.float32, name="dg", tag="dg")
        nc.vector.tensor_tensor(dg[:], lg3[:, :, 0], lg3[:, :, 1], AL.subtract)
        zs = work.tile([128, NT], dt.float32, name="zs", tag="dg2")
        nc.vector.tensor_tensor(zs[:], pz[:], dg[:], AL.add)
        rel = big.tile([128, NT], dt.float32, name="rel")
        nc.scalar.activation(rel[:], zs[:], AF.Sigmoid, scale=1.0 / TAU)
        for t in range(NT):
            z = tsz(t)
            nc.vector.tensor_scalar(srel[:z, t, :], srel[:z, t, :], rel[:z, t:t + 1],
                                    None, AL.mult)

        # ---- decoder precompute: ir/ii/in projections of all teacher inputs ----
        gpre = []
        for g in range(3):
            pre = big.tile([128, TS * R], dt.float32, name=f"gpre{g}")
            for q in range(4):
                q0 = q * 380
                pq = ps.tile([128, 380], dt.float32, name=nm("pq"), tag="pgen")
                nc.tensor.matmul(pq[:], wir[:, g * 128:(g + 1) * 128],
                                 insall[:, q0:q0 + 380], start=True, stop=True)
                nc.vector.tensor_copy(pre[:, q0:q0 + 380], pq[:])
            gpre.append(pre)
        irpre, iipre, inpre = gpre

        hidden = big.tile([128, R], dt.float32, name="hidden")
        nc.vector.memset(hidden[:], 0.0)
        predsb = big.tile([D, TS * R], dt.float32, name="predsb")

        # ================= decoder =================
        for st in range(TS):
            o0 = st * R
            ins_t = insall[:, o0:o0 + R]
            puvd = ps.tile([R, 256], dt.float32, name=nm("puvd"), tag="pgen")
            nc.tensor.matmul(puvd[:], hidden[:],
                             wmat[:, WM["msga"] * 128:(WM["msgb"] + 1) * 128],
                             start=True, stop=True)
            uvd = bige.tile([R, 256], dt.float32, name=nm("uvd"), tag="uvd")
            nc.vector.tensor_copy(uvd[:], puvd[:])

            tanh1 = bige.tile([128, EL], dt.float32, name=nm("tanh1"), tag="tanh1")
            for ci in range(NCH):
                c0 = ci * CH
                pe = pse.tile([128, CH], dt.float32, name=nm("ped"), tag="pe")
                nc.tensor.matmul(pe[:], uvd[:, 0:128], grec[:, c0:c0 + CH], start=True, stop=False)
                nc.tensor.matmul(pe[:], uvd[:, 128:256], gsend[:, c0:c0 + CH], start=False, stop=True)
                nc.scalar.activation(tanh1[:, c0:c0 + CH], pe[:], AF.Tanh, bias=cv("msgb1"))

            msgsT = bige.tile([128, NT, 128], dt.float32, name=nm("msgsT"), tag="msgsT")
            for t in range(NT):
                z = tsz(t)
                pt2 = ps.tile([128, 128], dt.float32, name=nm("pt2"), tag="pgen")
                nc.tensor.matmul(pt2[:z, :], ones1[0:1, 0:z], b2row[:], start=True, stop=False)
                nc.tensor.matmul(pt2[:z, :], tanh1[:, t * 128:t * 128 + z], wm("msgw2"),
                                 start=False, stop=True)
                nc.scalar.activation(msgsT[:z, t, :], pt2[:z, :], AF.Tanh)

            pagg = ps.tile([128, R], dt.float32, name=nm("pagg"), tag="pgen")
            for t in range(NT):
                z = tsz(t)
                nc.tensor.matmul(pagg[:], msgsT[:z, t, :], srel[:z, t, :],
                                 start=(t == 0), stop=(t == NT - 1))
            agg = work.tile([128, R], dt.float32, name=nm("agg"), tag="agg")
            nc.vector.tensor_copy(agg[:], pagg[:])

            pr = ps.tile([128, R], dt.float32, name=nm("pr"), tag="pgen")
            nc.tensor.matmul(pr[:], ident[:], irpre[:, o0:o0 + R], start=True, stop=False)
            nc.tensor.matmul(pr[:], wm("hr"), agg[:], start=False, stop=True)
            r_sb = work.tile([128, R], dt.float32, name=nm("rsb"), tag="rsb")
            nc.scalar.activation(r_sb[:], pr[:], AF.Sigmoid, bias=cv("bir"))

            pi = ps.tile([128, R], dt.float32, name=nm("pi"), tag="pgen")
            nc.tensor.matmul(pi[:], ident[:], iipre[:, o0:o0 + R], start=True, stop=False)
            nc.tensor.matmul(pi[:], wm("hi"), agg[:], start=False, stop=True)
            i_sb = work.tile([128, R], dt.float32, name=nm("isb"), tag="isb")
            nc.scalar.activation(i_sb[:], pi[:], AF.Sigmoid, bias=cv("bii"))

            phn = ps.tile([128, R], dt.float32, name=nm("phn"), tag="pgen")
            nc.tensor.matmul(phn[:], wm("hn"), agg[:], start=True, stop=True)
            rn = work.tile([128, R], dt.float32, name=nm("rn"), tag="rn")
            nc.vector.tensor_tensor(rn[:], r_sb[:], phn[:], AL.mult)
            pnn = ps.tile([128, R], dt.float32, name=nm("pnn"), tag="pgen")
            nc.tensor.matmul(pnn[:], ident[:], inpre[:, o0:o0 + R], start=True, stop=False)
            nc.tensor.matmul(pnn[:], ident[:], rn[:], start=False, stop=True)
            n_sb = work.tile([128, R], dt.float32, name=nm("nsb"), tag="nsb")
            nc.scalar.activation(n_sb[:], pnn[:], AF.Tanh, bias=cv("bin"))

            tmp = work.tile([128, R], dt.float32, name=nm("tmp"), tag="tmp")
            nc.vector.tensor_tensor(tmp[:], hidden[:], n_sb[:], AL.subtract)
            nc.vector.tensor_tensor(tmp[:], i_sb[:], tmp[:], AL.mult)
            nc.vector.tensor_tensor(hidden[:], n_sb[:], tmp[:], AL.add)

            po1 = ps.tile([128, R], dt.float32, name=nm("po1"), tag="pgen")
            nc.tensor.matmul(po1[:], wm("of1"), hidden[:], start=True, stop=True)
            o1 = work.tile([128, R], dt.float32, name=nm("o1"), tag="o1")
            nc.scalar.activation(o1[:], po1[:], AF.Relu, bias=cv("bof1"))
            po2 = ps.tile([128, R], dt.float32, name=nm("po2"), tag="pgen")
            nc.tensor.matmul(po2[:], wm("of2"), o1[:], start=True, stop=True)
            o2 = work.tile([128, R], dt.float32, name=nm("o2"), tag="o2")
            nc.scalar.activation(o2[:], po2[:], AF.Relu, bias=cv("bof2"))
            pp = ps.tile([D, R], dt.float32, name=nm("pp"), tag="pgen")
            nc.tensor.matmul(pp[:], wof3[:], o2[:], start=True, stop=False)
            nc.tensor.matmul(pp[:], ident[0:D, 0:D], ins_t, start=False, stop=True)
            nc.scalar.activation(predsb[:, o0:o0 + R], pp[:], AF.Identity, bias=bof3[:])

        nc.sync.dma_start(out_t[:], predsb[:])
        ctx.close()

    nc.compile()
    return nc


def _prep_inputs(data, gumbel_u, params, rec_idx, send_idx, db):
    data = _f32(data)
    gum = _f32(gumbel_u)
    rec = np.asarray(rec_idx).astype(np.int64)
    send = np.asarray(send_idx).astype(np.int64)
    p = {k: ({kk: _f32(vv) for kk, vv in v.items()} if isinstance(v, dict) else _f32(v))
         for k, v in params.items()}

    cvec = np.zeros((128, 27), np.float32)
    cvec[:, 23] = -1.0
    cvec[:, 24] = BN_EPS
    cvec[:, 25] = GEPS

    def setc(name, vec):
        v = np.asarray(vec, np.float32).ravel()
        cvec[:len(v), CV[name]] = v

    for i, mk in enumerate(["mlp1", "mlp2", "mlp3", "mlp4"]):
        mp = p[mk]
        setc(f"m{i + 1}b1", mp["b1"] + 1.0)
        setc(f"m{i + 1}b2", mp["b2"] - mp["w2"].sum(axis=1) + 1.0)
        setc(f"g{i + 1}", mp["g"])
        setc(f"bb{i + 1}", mp["bb"])
    setc("msgb1", p["msg1_b"][1])
    setc("bir", p["ir_b"])
    setc("bii", p["ii_b"])
    setc("bin", p["in_b"])
    setc("bof1", p["of1_b"])
    setc("bof2", p["of2_b"])
    setc("dwt", p["fc_out_w"][1] - p["fc_out_w"][0])

    wmat = np.zeros((128, NWM * 128), np.float32)

    def setw(name, w):
        wmat[:w.shape[1], WM[name] * 128:WM[name] * 128 + w.shape[0]] = w.T

    setw("m1w2", p["mlp1"]["w2"])
    setw("m2a", p["mlp2"]["w1"][:, :128]); setw("m2b", p["mlp2"]["w1"][:, 128:])
    setw("m2w2", p["mlp2"]["w2"])
    setw("m3w1", p["mlp3"]["w1"]); setw("m3w2", p["mlp3"]["w2"])
    setw("m4a", p["mlp4"]["w1"][:, :128]); setw("m4b", p["mlp4"]["w1"][:, 128:256])
    setw("m4c", p["mlp4"]["w1"][:, 256:]); setw("m4w2", p["mlp4"]["w2"])
    setw("msga", p["msg1_w"][1][:, :128]); setw("msgb", p["msg1_w"][1][:, 128:])
    setw("msgw2", p["msg2_w"][1])
    setw("hr", p["hr_w"]); setw("hi", p["hi_w"]); setw("hn", p["hn_w"])
    setw("of1", p["of1_w"]); setw("of2", p["of2_w"])

    wm1 = np.ascontiguousarray(p["mlp1"]["w1"].T)           # [80,128]
    wir = np.concatenate([p["ir_w"].T, p["ii_w"].T, p["in_w"].T], axis=1)  # [4,384]
    wof3 = np.ascontiguousarray(p["of3_w"].T)               # [128,4]
    bof3 = p["of3_b"].reshape(D, 1)
    b2row = p["msg2_b"][1].reshape(1, 128)

    # one-hot gather / scatter matrices
    grec = np.zeros((R, EL), np.float32)
    gsend = np.zeros((R, EL), np.float32)
    for b in range(BL):
        for e in range(E):
            grec[b * N + rec[e], b * E + e] = 1.0
            gsend[b * N + send[e], b * E + e] = 1.0
    s01n = np.zeros((128, NT, R), np.float32)
    s01d = np.zeros((128, NT, R), np.float32)
    for e in range(EL):
        t, pp_ = e // 128, e % 128
        tgt = (e // E) * N + rec[e % E]
        s01n[pp_, t, tgt] = 1.0 / N
        s01d[pp_, t, tgt] = 1.0 / D

    in_maps = []
    for core in range(NC):
        b0 = core * BL
        dloc = data[b0:b0 + BL]                       # [2,40,20,4]
        x1 = dloc.reshape(BL * N, T * D).T.copy()     # [80, 80] feature t*4+d
        ins = dloc[:, :, :TS].transpose(3, 2, 0, 1).reshape(D, TS, BL * N)
        insall = np.ascontiguousarray(ins.reshape(D, TS * R))
        gml = np.full((128, NT, 2), 0.5, np.float32)
        gl = gum[b0:b0 + BL].reshape(EL, 2)
        for e in range(EL):
            gml[e % 128, e // 128] = gl[e]
        in_maps.append(dict(
            x1=x1, insall=insall, gum=gml, grec=grec, gsend=gsend,
            s01n=s01n, s01d=s01d, cvec=cvec, wmat=wmat, wm1=wm1, wir=wir,
            wof3=wof3, bof3=bof3, b2row=b2row))
    return in_maps


def kernel(data, gumbel_u, params, rec_idx, send_idx):
    import os
    from concourse.bass_utils import run_bass_kernel_spmd
    if "nc" not in _CACHE:
        _CACHE["nc"] = _build()
    nc = _CACHE["nc"]

    # fc_out delta-bias enters through the sigmoid's scalar bias on host side:
    # sigmoid((dz + dg + db)/TAU). We add db into gumbel's dg? Simpler: fold db
    # into cvec "dwt"? Not possible (it multiplies x4). Instead add db to dz by
    # shifting dg: dg' = dg + db is wrong per-element? db is constant -> add to
    # gum-derived dg on device is equivalent to adding to zsum; we instead add
    # db/TAU at sigmoid time via scale trick -- handled here by adjusting the
    # gumbel wrap: we pre-add db to one lg term is nonlinear. So: we add db by
    # padding cvec col unused... final: add db to dz via extra column is
    # omitted; instead we adjust gumbel input u1 so that dg absorbs db exactly:
    # not possible. => handled below by adding db to dg via kernel input 'gum'?
    # We choose the exact route: adjust host 'dwt' path is wrong; we instead
    # exploit sigmoid bias being an immediate in the built program: db was
    # unknown at build time, so we instead add db directly to the teacher
    # logits by shifting x4 @ dwt with a per-edge constant... see note in
    # _prep_inputs: we bake db into s01d? No -- we bake it into gum below.
    p = params
    db = float(np.asarray(p["fc_out_b"])[1] - np.asarray(p["fc_out_b"])[0])
    in_maps = _prep_inputs(data, gumbel_u, params, rec_idx, send_idx, db)
    # Exact db handling: dg_dev = t2_0 - t2_1 where t2_k = ln(GEPS - ln(u_k+GEPS)).
    # We need zsum = dz + dg + db. Replace u_0 by u_0' such that
    # t2_0' = t2_0 + db  =>  ln(GEPS - ln(u0'+GEPS)) = t2_0 + db
    # =>  u0' = exp(GEPS - exp(t2_0 + db)) - GEPS, computed on host from u0.
    for im in in_maps:
        g = im["gum"]
        u0 = g[:, :, 0].astype(np.float64)
        t20 = np.log(GEPS - np.log(u0 + GEPS))
        u0p = np.exp(GEPS - np.exp(t20 + db)) - GEPS
        g[:, :, 0] = u0p.astype(np.float32)

    res = run_bass_kernel_spmd(nc, in_maps, list(range(NC)),
                               trace=bool(os.environ.get("BASS_TRACE_RES")))
    _CACHE["last_res"] = res
    out = np.zeros((B, N, TS, D), np.float32)
    for core in range(NC):
        arr = res.results[core]["preds"]              # [4, 19*80]
        a = arr.reshape(D, TS, BL, N).transpose(2, 3, 1, 0)
        out[core * BL:(core + 1) * BL] = a
    return out
